# revision 18
# baseline (speedup 1.0000x reference)
"""BERT self-attention (B=4, S=2048, H=768, 12 heads x 64) on 8 trn2 cores.

Sharding: core c = batch (c//2) x head-half (c%2, 6 heads each).
Each core computes Q/K/V projections for its 6 heads, attention, and a
partial output projection (its heads' slice of Wo). Host sums the two
partials per batch and adds bo.

On-device layout (per core):
  xT   [768, 2048]  bf16  (host-transposed x), 4 pieces of 512 per chunk
  QT/KT per head-pair [128=2x64, 2048] bf16   (head-dim on partitions)
  V    16 tiles [128 keys, 6 heads x 65] bf16 (col 0 of each head = 1.0)
  scores^T [128 keys, 2x512 q] fp32 PSUM, two heads concurrent via PE
  row tiling (row_grp h0/h64, shared 512-col rhs stream)
  exp on ScalarE (scale=1/8, bias=mask column), out bf16
  attn@V -> cb [65, 512] PSUM; row 0 = softmax denominator
  combT packed [128, 2048] bf16 per head pair, scaled by 1/denom
  out-proj: single K=128 matmul per (st, half); out_acc bf16, out bf16

The attention loop is ACT(exp)-bound (~1.11us per 128x1024 exp). The
in-order PE is kept fed by (a) pipelining the score matmuls one slot
ahead and (b) injecting independent projection matmuls into the
exp-wait bubble between scores(i+1) and attn@V(i). Startup DMAs issue
on the sync+gpsimd queues (scalar queue stays clear so exp(0) is not
delayed behind DMA issue costs), ordered so the hp0 K/Q chains start
as early as possible.
"""

import numpy as np
import ml_dtypes

B, S, H = 4, 2048, 768
NH, HS = 12, 64
NHL = 6              # heads per core
NHP = 3              # head pairs per core
HCHUNKS = 6          # 768 / 128 contraction chunks
SKT = 16             # key tiles of 128
SQT = 4              # query tiles of 512
QW = 512             # query tile width
N_CORES = 8

_COMPILED = None


def _build():
    import concourse.bass as bass
    import concourse.mybir as mybir
    import concourse.tile as tile
    from concourse import bacc

    fp32 = mybir.dt.float32
    bf16 = mybir.dt.bfloat16
    AF = mybir.ActivationFunctionType

    nc = bacc.Bacc("TRN2", target_bir_lowering=False, debug=False)

    xt_d = nc.dram_tensor("xt", [H, S], bf16, kind="ExternalInput").ap()
    wq_d = nc.dram_tensor("wq", [H, NHL * HS], bf16, kind="ExternalInput").ap()
    wk_d = nc.dram_tensor("wk", [H, NHL * HS], bf16, kind="ExternalInput").ap()
    wv_d = nc.dram_tensor("wv", [H, NHL * HS], bf16, kind="ExternalInput").ap()
    wo_d = nc.dram_tensor("wo", [NHL * HS, H], bf16, kind="ExternalInput").ap()
    bq_d = nc.dram_tensor("bq", [128, NHP], fp32, kind="ExternalInput").ap()
    bk_d = nc.dram_tensor("bk", [128, NHP], fp32, kind="ExternalInput").ap()
    bv_d = nc.dram_tensor("bv", [128, NHL * HS], bf16, kind="ExternalInput").ap()
    mask_d = nc.dram_tensor("mask", [128, SKT], fp32, kind="ExternalInput").ap()
    out_d = nc.dram_tensor("out", [S, H], bf16, kind="ExternalOutput").ap()

    with tile.TileContext(nc) as tc:
        with (
            tc.tile_pool(name="const", bufs=1) as const,
            tc.tile_pool(name="xt", bufs=1) as xtp,
            tc.tile_pool(name="vsb", bufs=1) as vsb,
            tc.tile_pool(name="qkt", bufs=2) as qkt,
            tc.tile_pool(name="combt", bufs=1) as combtp,
            tc.tile_pool(name="oacc", bufs=1) as oaccp,
            tc.tile_pool(name="attn", bufs=5) as attnp,
            tc.tile_pool(name="small", bufs=4) as smallp,
            tc.tile_pool(name="ps_sc", bufs=2, space="PSUM") as ps_sc,
            tc.tile_pool(name="ps_cb", bufs=2, space="PSUM") as ps_cb,
            tc.tile_pool(name="ps_pj", bufs=2, space="PSUM") as ps_pj,
        ):
            # ---- startup DMAs in consumption order. The hp0 K chain
            # needs (xt piece0, wk), the Q chain wq; V needs wv + xt
            # piece1. Issues cost ~650ns of queue time each and pace
            # the transfers, so split by need-time: sync carries
            # xt0/wv/xt2/xt3/wo, scalar carries wk/wq/biases/xt1 and
            # clears its backlog (~14us) before the first exp. ----
            xt_t = [xtp.tile([128, HCHUNKS, QW], bf16, tag=f"xt{p}",
                             name=f"xt{p}") for p in range(SQT)]
            wk_t = const.tile([128, HCHUNKS, NHL * HS], bf16, tag="wk")
            wq_t = const.tile([128, HCHUNKS, NHL * HS], bf16, tag="wq")
            wv_t = const.tile([128, HCHUNKS, NHL * HS], bf16, tag="wv")
            bq_sb = const.tile([128, NHP], fp32, tag="bq")
            bk_sb = const.tile([128, NHP], fp32, tag="bk")
            bv_sb = const.tile([128, NHL * HS], bf16, tag="bv")
            mask_sb = const.tile([128, SKT], fp32, tag="mask")
            wo_sb = [const.tile([128, H], bf16, tag=f"wo{hp}", name=f"wo{hp}")
                     for hp in range(NHP)]

            def ld_xt(piece, q):
                for c in range(HCHUNKS):
                    q.dma_start(
                        xt_t[piece][:, c, :],
                        xt_d[c * 128:(c + 1) * 128,
                             piece * QW:(piece + 1) * QW])

            def ld_w(dst, srcap, q):
                for c in range(HCHUNKS):
                    q.dma_start(dst[:, c, :],
                                srcap[c * 128:(c + 1) * 128, :])

            # (DMA issues pace at ring-completion rate ~1.3us/transfer
            # per queue, so scalar carries only ~7us of critical weight
            # traffic — it must be free to run exp(0) — and the
            # latest-needed bulk rides the gpsimd SWDGE queue.)
            ld_xt(0, nc.sync)
            ld_w(wk_t, wk_d, nc.scalar)
            nc.scalar.dma_start(bk_sb[:], bk_d[:])
            ld_w(wq_t, wq_d, nc.scalar)
            nc.scalar.dma_start(bq_sb[:], bq_d[:])
            nc.scalar.dma_start(mask_sb[:], mask_d[:])
            nc.scalar.dma_start(bv_sb[:], bv_d[:])
            ld_w(wv_t, wv_d, nc.sync)
            ld_xt(1, nc.sync)
            ld_xt(2, nc.gpsimd)
            ld_xt(3, nc.gpsimd)
            for hp in range(NHP):
                nc.gpsimd.dma_start(wo_sb[hp][:],
                                    wo_d[hp * 128:(hp + 1) * 128, :])

            # ---- V projection: V[s, h*65 + 1 + d], col h*65+0 = 1.0
            # (denominator lands on PSUM partition 0 of the attn@V
            # output, where reciprocal can read it directly).
            # Emitted as per-kt unit chains so the tail can be injected
            # into the attention loop. ----
            v_sb = [vsb.tile([128, NHL, HS + 1], bf16, tag=f"v{kt}", name=f"v{kt}")
                    for kt in range(SKT)]

            def v_units(kt):
                vt = v_sb[kt]
                st8 = {}

                def unit(c, st8=st8):
                    if c == 0:
                        st8["ps"] = ps_pj.tile([128, 512], fp32, tag="pj",
                                               name="psv")
                    nc.tensor.matmul(
                        st8["ps"][:, :NHL * HS],
                        lhsT=xt_t[kt // 4][:, c,
                                           (kt % 4) * 128:(kt % 4 + 1) * 128],
                        rhs=wv_t[:, c, :],
                        start=(c == 0),
                        stop=(c == HCHUNKS - 1),
                    )
                    if c == HCHUNKS - 1:
                        nc.vector.tensor_add(
                            vt[:, :, 0:HS],
                            st8["ps"][:, :NHL * HS].rearrange(
                                "p (h d) -> p h d", h=NHL),
                            bv_sb[:].rearrange("p (h d) -> p h d", h=NHL),
                        )
                        nc.vector.memset(vt[:, :, HS:HS + 1], 1.0)

                return [lambda c=c: unit(c) for c in range(HCHUNKS)]

            combt = [combtp.tile([128, S], bf16, tag=f"ct{hp}", name=f"ct{hp}")
                     for hp in range(NHP)]
            # partial output accumulator [128, st, 768] (bf16: host sums
            # the two per-batch partials in fp32)
            out_acc = oaccp.tile([128, S // 128, H], bf16, tag="oacc")

            def emit_qkt(hp):
                """Q^T/K^T projection for head pair hp; returns (qt, kt, units).
                Each unit emits one matmul (plus bias-add drain on the last)."""
                qt_t = qkt.tile([128, S], bf16, tag="qt", name=f"qt{hp}")
                kt_t = qkt.tile([128, S], bf16, tag="kt", name=f"kt{hp}")
                units = []
                chains = {}
                for kind, dst, w_t, b_sb in (("kt", kt_t, wk_t, bk_sb),
                                             ("qt", qt_t, wq_t, bq_sb)):
                    for sq in range(SQT):
                        st8 = {}

                        def unit(c, dst=dst, w_t=w_t, b_sb=b_sb, sq=sq, st8=st8):
                            if c == 0:
                                st8["ps"] = ps_pj.tile(
                                    [128, 512], fp32, tag="pj", name="psq")
                            nc.tensor.matmul(
                                st8["ps"][:],
                                lhsT=w_t[:, c, hp * 128:(hp + 1) * 128],
                                rhs=xt_t[sq][:, c, :],
                                start=(c == 0),
                                stop=(c == HCHUNKS - 1),
                            )
                            if c == HCHUNKS - 1:
                                nc.vector.tensor_scalar_add(
                                    dst[:, sq * QW:(sq + 1) * QW], st8["ps"][:],
                                    b_sb[:, hp:hp + 1],
                                )

                        chain = [lambda c=c, u=unit: u(c)
                                 for c in range(HCHUNKS)]
                        chains[(kind, sq)] = chain
                        units.extend(chain)
                return qt_t, kt_t, units, chains

            def emit_outproj_unit(hp, st, half, stream_out=False):
                """Single K=128 out-proj matmul for (head pair hp, s-tile
                st, 384-col half), contracting both heads at once, plus
                the accumulate drain."""
                ps = ps_pj.tile([128, 512], fp32, tag="pj", name="pso")
                nc.tensor.matmul(
                    ps[:, 0:384],
                    lhsT=combt[hp][:, st * 128:(st + 1) * 128],
                    rhs=wo_sb[hp][:, half * 384:(half + 1) * 384],
                    start=True, stop=True,
                )
                dst = out_acc[:, st, half * 384:(half + 1) * 384]
                if hp == 0:
                    nc.vector.tensor_copy(dst, ps[:, 0:384])
                else:
                    nc.vector.tensor_add(dst, dst, ps[:, 0:384])
                if stream_out:
                    nc.sync.dma_start(
                        out_d[st * 128:(st + 1) * 128, :], out_acc[:, st, :])

            def outproj_units(hp, sqs, stream_out=False, min_sq=None):
                """(min_sq, unit) out-proj work for the s-tiles inside query
                tiles `sqs` of head pair hp, gated one sq later (or at an
                explicit min_sq when queued into a later head pair)."""
                units = []
                for sq in sqs:
                    gate = sq + 2 if min_sq is None else min_sq
                    for st in range(4 * sq, 4 * (sq + 1)):
                        for half in range(2):
                            units.append((gate, lambda hp=hp, st=st, half=half,
                                          so=stream_out and half == 1:
                                          emit_outproj_unit(hp, st, half, so)))
                return units

            slots = [(hp, sq, kt) for hp in range(NHP) for sq in range(SQT)
                     for kt in range(SKT)]

            def scores(hp, sq, kt):
                qt_t, kt_t = qkts[hp][0], qkts[hp][1]
                sc = ps_sc.tile([128, 1024], fp32, tag="sc", name="sc")
                nc.tensor.matmul(
                    sc[:, 0:512],
                    lhsT=kt_t[0:64, kt * 128:(kt + 1) * 128],
                    rhs=qt_t[0:64, sq * QW:(sq + 1) * QW],
                    start=True, stop=True,
                )
                nc.tensor.matmul(
                    sc[:, 512:1024],
                    lhsT=kt_t[64:128, kt * 128:(kt + 1) * 128],
                    rhs=qt_t[64:128, sq * QW:(sq + 1) * QW],
                    start=True, stop=True,
                )
                return sc

            # ---- pre-loop: hp0 K/Q chains for sq0, first scores, V kt0
            # (the rest of V is drained slot-by-slot with a one-slot
            # lookahead so every v tile's writers are emitted before the
            # attn@V matmul that reads it — Tile orders by program
            # order, so a late-emitted writer is a silent race) ----
            qkts = [emit_qkt(0)]
            ch0 = qkts[0][3]
            for u in ch0[("kt", 0)] + ch0[("qt", 0)]:
                u()
            sc_cur = scores(*slots[0])
            for u in v_units(0):
                u()
            for u in ch0[("kt", 1)]:
                u()
            v_pending = {kt: v_units(kt) for kt in range(1, SKT)}

            # per-hp injection queues: (min_sq, emit_fn)
            inject_q = {0: [], 1: [], 2: []}
            qkts.append(emit_qkt(1))
            inject_q[0] = (
                [(0, u) for u in ch0[("kt", 2)]]
                + [(0, u) for u in ch0[("kt", 3)]]
                + [(0, u) for u in ch0[("qt", 1)]]
                + [(1, u) for u in ch0[("qt", 2)]]
                + [(2, u) for u in ch0[("qt", 3)]]
                + [(0, u) for u in qkts[1][2]]
                + outproj_units(0, range(SQT - 2)))

            cb_cur = None
            for i, (hp, sq, kt) in enumerate(slots):
                if kt == 0:
                    if sq == 0 and hp > 0:
                        # drain any leftover injected work of the previous hp
                        for _, u in inject_q[hp - 1]:
                            u()
                        inject_q[hp - 1] = []
                    # build hp-level injection queues lazily at hp start
                    if sq == 0 and hp == 1:
                        qkts.append(emit_qkt(2))
                        inject_q[1] = (outproj_units(0, [SQT - 2, SQT - 1], min_sq=0)
                                       + [(0, u) for u in qkts[2][2]]
                                       + outproj_units(1, range(SQT - 2)))

                    if sq == 0 and hp == 2:
                        inject_q[2] = (outproj_units(1, [SQT - 2, SQT - 1], min_sq=0)
                                       + outproj_units(2, range(SQT - 2),
                                                       stream_out=True)
                                       + outproj_units(2, [SQT - 2],
                                                       stream_out=True,
                                                       min_sq=SQT - 1))
                    cb_a = ps_cb.tile([65, 512], fp32, tag="cb", name="cba")
                    cb_b = ps_cb.tile([65, 512], fp32, tag="cb", name="cbb")
                    cb_cur = (cb_a, cb_b)
                # lookahead scores for the next slot
                sc_nxt = scores(*slots[i + 1]) if i + 1 < len(slots) else None
                at = attnp.tile([128, 1024], bf16, tag="at")
                nc.scalar.activation(
                    at[:], sc_cur[:], AF.Exp,
                    bias=mask_sb[:, kt:kt + 1], scale=0.125,
                )
                # during hp0-sq0, emit next slot's V projection (writers
                # must precede the attn@V reader in program order)
                if hp == 0 and sq == 0 and kt + 1 in v_pending:
                    for u in v_pending.pop(kt + 1):
                        u()
                # fill the PE exp-wait bubble with independent work
                # (scan past gated units so a blocked head doesn't starve
                # eligible work behind it)
                q = inject_q[hp]
                popped = 0
                max_pop = 2 if hp == 0 else (3 if (hp == 1 and sq == 0) else 2)
                j = 0
                while j < len(q) and popped < max_pop:
                    if q[j][0] <= sq:
                        q.pop(j)[1]()
                        popped += 1
                    else:
                        j += 1
                cb_a, cb_b = cb_cur
                nc.tensor.matmul(
                    cb_a[:],
                    lhsT=v_sb[kt][:, 2 * hp, :],
                    rhs=at[:, 0:512],
                    start=(kt == 0), stop=(kt == SKT - 1),
                )
                nc.tensor.matmul(
                    cb_b[:],
                    lhsT=v_sb[kt][:, 2 * hp + 1, :],
                    rhs=at[:, 512:1024],
                    start=(kt == 0), stop=(kt == SKT - 1),
                )
                sc_cur = sc_nxt
                if kt == SKT - 1:
                    # normalize: comb rows 0..63 / denom (row 64).
                    # Two quick copies free both PSUM banks before the
                    # slow recip/broadcast chains run.
                    cbs_list = []
                    for cb in (cb_a, cb_b):
                        cbs = smallp.tile([65, 512], fp32, tag="cbs", name="cbs")
                        nc.vector.tensor_copy(cbs[:], cb[:])
                        cbs_list.append(cbs)
                    for half, cbs in ((0, cbs_list[0]), (1, cbs_list[1])):
                        rc0 = smallp.tile([1, 512], fp32, tag="rc0")
                        nc.sync.dma_start(rc0[:], cbs[64:65, :])
                        rc1 = smallp.tile([1, 512], fp32, tag="rc1")
                        # approx recip is partition-0 only on HW
                        nc.vector.reciprocal_approx_fast(rc1[:], rc0[:])
                        bc = smallp.tile([64, 512], fp32, tag="bc")
                        nc.gpsimd.partition_broadcast(bc[:], rc1[:])
                        nc.vector.tensor_mul(
                            combt[hp][64 * half:64 * (half + 1),
                                      sq * QW:(sq + 1) * QW],
                            cbs[0:64, :], bc[:],
                        )

            # ---- tail: leftovers (hp2 out-proj of sq3), streaming out ----
            for hp in range(NHP):
                for _, u in inject_q[hp]:
                    u()
                inject_q[hp] = []
            for st in range(4 * (SQT - 1), 4 * SQT):
                for half in range(2):
                    emit_outproj_unit(2, st, half, stream_out=(half == 1))

    nc.compile()
    return nc


def _get_compiled():
    global _COMPILED
    if _COMPILED is None:
        _COMPILED = _build()
    return _COMPILED


def _prep_core_inputs(x, mask, Wq, bq, Wk, bk, Wv, bv, Wo, core):
    b, hg = core // 2, core % 2
    lo, hi = hg * NHL * HS, (hg + 1) * NHL * HS
    bf = ml_dtypes.bfloat16
    return {
        "xt": np.ascontiguousarray(x[b].T).astype(bf),
        "wq": np.ascontiguousarray(Wq[:, lo:hi]).astype(bf),
        "wk": np.ascontiguousarray(Wk[:, lo:hi]).astype(bf),
        "wv": np.ascontiguousarray(Wv[:, lo:hi]).astype(bf),
        "wo": np.ascontiguousarray(Wo[lo:hi, :]).astype(bf),
        "bq": np.ascontiguousarray(bq[lo:hi].reshape(NHP, 128).T).astype(np.float32),
        "bk": np.ascontiguousarray(bk[lo:hi].reshape(NHP, 128).T).astype(np.float32),
        "bv": np.tile(bv[lo:hi][None, :], (128, 1)).astype(bf),
        "mask": np.ascontiguousarray(
            mask[b, 0, 0].reshape(SKT, 128).T).astype(np.float32),
    }


def kernel(x, additive_attention_mask, Wq, bq, Wk, bk, Wv, bv, Wo, bo):
    from concourse import bass2jax

    x = np.asarray(x, dtype=np.float32)
    mask = np.asarray(additive_attention_mask, dtype=np.float32)
    args = [np.asarray(a, dtype=np.float32) for a in (Wq, bq, Wk, bk, Wv, bv, Wo)]
    Wq, bq, Wk, bk, Wv, bv, Wo = args
    bo = np.asarray(bo, dtype=np.float32)

    nc = _get_compiled()
    in_maps = [
        _prep_core_inputs(x, mask, Wq, bq, Wk, bk, Wv, bv, Wo, c)
        for c in range(N_CORES)
    ]
    results = bass2jax.run_bass_via_pjrt(nc, in_maps, n_cores=N_CORES)

    out = np.empty((B, S, H), dtype=np.float32)
    for b in range(B):
        out[b] = (results[2 * b]["out"].astype(np.float32)
                  + results[2 * b + 1]["out"].astype(np.float32) + bo)
    return out


# revision 21
# speedup vs baseline: 1.0046x; 1.0046x over previous
"""BERT self-attention (B=4, S=2048, H=768, 12 heads x 64) on 8 trn2 cores.

Sharding: core c = batch (c//2) x head-half (c%2, 6 heads each).
Each core computes Q/K/V projections for its 6 heads, attention, and a
partial output projection (its heads' slice of Wo). Host sums the two
partials per batch and adds bo.

On-device layout (per core):
  xT   [768, 2048]  bf16  (host-transposed x), 4 pieces of 512 per chunk
  QT/KT per head-pair [128=2x64, 2048] bf16   (head-dim on partitions)
  V    16 tiles [128 keys, 6 heads x 65] bf16 (col 0 of each head = 1.0)
  scores^T [128 keys, 2x512 q] fp32 PSUM, two heads concurrent via PE
  row tiling (row_grp h0/h64, shared 512-col rhs stream)
  exp on ScalarE (scale=1/8, bias=mask column), out bf16
  attn@V -> cb [65, 512] PSUM; row 0 = softmax denominator
  combT packed [128, 2048] bf16 per head pair, scaled by 1/denom
  out-proj: single K=128 matmul per (st, half); out_acc bf16, out bf16

The attention loop is ACT(exp)-bound (~1.11us per 128x1024 exp). The
in-order PE is kept fed by (a) pipelining the score matmuls one slot
ahead and (b) injecting independent projection matmuls into the
exp-wait bubble between scores(i+1) and attn@V(i). Startup DMAs issue
on the sync+gpsimd queues (scalar queue stays clear so exp(0) is not
delayed behind DMA issue costs), ordered so the hp0 K/Q chains start
as early as possible.
"""

import numpy as np
import ml_dtypes

B, S, H = 4, 2048, 768
NH, HS = 12, 64
NHL = 6              # heads per core
NHP = 3              # head pairs per core
HCHUNKS = 6          # 768 / 128 contraction chunks
SKT = 16             # key tiles of 128
SQT = 4              # query tiles of 512
QW = 512             # query tile width
N_CORES = 8

_COMPILED = None


def _build():
    import concourse.bass as bass
    import concourse.mybir as mybir
    import concourse.tile as tile
    from concourse import bacc

    fp32 = mybir.dt.float32
    bf16 = mybir.dt.bfloat16
    AF = mybir.ActivationFunctionType

    nc = bacc.Bacc("TRN2", target_bir_lowering=False, debug=False)

    xt_d = nc.dram_tensor("xt", [H, S], bf16, kind="ExternalInput").ap()
    wq_d = nc.dram_tensor("wq", [H, NHL * HS], bf16, kind="ExternalInput").ap()
    wk_d = nc.dram_tensor("wk", [H, NHL * HS], bf16, kind="ExternalInput").ap()
    wv_d = nc.dram_tensor("wv", [H, NHL * HS], bf16, kind="ExternalInput").ap()
    wo_d = nc.dram_tensor("wo", [NHL * HS, H], bf16, kind="ExternalInput").ap()
    bq_d = nc.dram_tensor("bq", [128, NHP], fp32, kind="ExternalInput").ap()
    bk_d = nc.dram_tensor("bk", [128, NHP], fp32, kind="ExternalInput").ap()
    bv_d = nc.dram_tensor("bv", [128, NHL * HS], bf16, kind="ExternalInput").ap()
    mask_d = nc.dram_tensor("mask", [128, SKT], fp32, kind="ExternalInput").ap()
    out_d = nc.dram_tensor("out", [S, H], bf16, kind="ExternalOutput").ap()

    with tile.TileContext(nc) as tc:
        with (
            tc.tile_pool(name="const", bufs=1) as const,
            tc.tile_pool(name="xt", bufs=1) as xtp,
            tc.tile_pool(name="vsb", bufs=1) as vsb,
            tc.tile_pool(name="qkt", bufs=2) as qkt,
            tc.tile_pool(name="combt", bufs=1) as combtp,
            tc.tile_pool(name="oacc", bufs=1) as oaccp,
            tc.tile_pool(name="attn", bufs=5) as attnp,
            tc.tile_pool(name="small", bufs=4) as smallp,
            tc.tile_pool(name="ps_sc", bufs=2, space="PSUM") as ps_sc,
            tc.tile_pool(name="ps_cb", bufs=2, space="PSUM") as ps_cb,
            tc.tile_pool(name="ps_pj", bufs=2, space="PSUM") as ps_pj,
        ):
            # ---- startup DMAs in consumption order. The hp0 K chain
            # needs (xt piece0, wk), the Q chain wq; V needs wv + xt
            # piece1. Issues cost ~650ns of queue time each and pace
            # the transfers, so split by need-time: sync carries
            # xt0/wv/xt2/xt3/wo, scalar carries wk/wq/biases/xt1 and
            # clears its backlog (~14us) before the first exp. ----
            xt_t = [xtp.tile([128, HCHUNKS, QW], bf16, tag=f"xt{p}",
                             name=f"xt{p}") for p in range(SQT)]
            wk_t = const.tile([128, HCHUNKS, NHL * HS], bf16, tag="wk")
            wq_t = const.tile([128, HCHUNKS, NHL * HS], bf16, tag="wq")
            wv_t = const.tile([128, HCHUNKS, NHL * HS], bf16, tag="wv")
            bq_sb = const.tile([128, NHP], fp32, tag="bq")
            bk_sb = const.tile([128, NHP], fp32, tag="bk")
            bv_sb = const.tile([128, NHL * HS], bf16, tag="bv")
            mask_sb = const.tile([128, SKT], fp32, tag="mask")
            wo_sb = [const.tile([128, H], bf16, tag=f"wo{hp}", name=f"wo{hp}")
                     for hp in range(NHP)]

            def ld_xt(piece, q):
                for c in range(HCHUNKS):
                    q.dma_start(
                        xt_t[piece][:, c, :],
                        xt_d[c * 128:(c + 1) * 128,
                             piece * QW:(piece + 1) * QW])

            def ld_w(dst, srcap, q):
                for c in range(HCHUNKS):
                    q.dma_start(dst[:, c, :],
                                srcap[c * 128:(c + 1) * 128, :])

            # (DMA transfers pace at ring-completion rate ~1.4us each
            # per queue, so the three critical tensors ride three
            # different queues, and wv is split 2-2-2 so the V chain
            # can start by ~12us.)
            ld_xt(0, nc.sync)
            ld_w(wk_t, wk_d, nc.scalar)
            nc.scalar.dma_start(bk_sb[:], bk_d[:])
            ld_w(wq_t, wq_d, nc.gpsimd)
            for c, q in ((0, nc.sync), (1, nc.sync), (2, nc.scalar),
                         (3, nc.scalar), (4, nc.gpsimd), (5, nc.gpsimd)):
                q.dma_start(wv_t[:, c, :], wv_d[c * 128:(c + 1) * 128, :])
            nc.scalar.dma_start(bq_sb[:], bq_d[:])
            nc.scalar.dma_start(mask_sb[:], mask_d[:])
            nc.scalar.dma_start(bv_sb[:], bv_d[:])
            ld_xt(1, nc.sync)
            for hp in range(NHP):
                nc.scalar.dma_start(wo_sb[hp][:],
                                    wo_d[hp * 128:(hp + 1) * 128, :])
            ld_xt(2, nc.gpsimd)
            ld_xt(3, nc.gpsimd)

            # ---- V projection: V[s, h*65 + 1 + d], col h*65+0 = 1.0
            # (denominator lands on PSUM partition 0 of the attn@V
            # output, where reciprocal can read it directly).
            # Emitted as per-kt unit chains so the tail can be injected
            # into the attention loop. ----
            v_sb = [vsb.tile([128, NHL, HS + 1], bf16, tag=f"v{kt}", name=f"v{kt}")
                    for kt in range(SKT)]

            def v_units(kt):
                vt = v_sb[kt]
                st8 = {}

                def unit(c, st8=st8):
                    if c == 0:
                        st8["ps"] = ps_pj.tile([128, 512], fp32, tag="pj",
                                               name="psv")
                    nc.tensor.matmul(
                        st8["ps"][:, :NHL * HS],
                        lhsT=xt_t[kt // 4][:, c,
                                           (kt % 4) * 128:(kt % 4 + 1) * 128],
                        rhs=wv_t[:, c, :],
                        start=(c == 0),
                        stop=(c == HCHUNKS - 1),
                    )
                    if c == HCHUNKS - 1:
                        nc.vector.tensor_add(
                            vt[:, :, 0:HS],
                            st8["ps"][:, :NHL * HS].rearrange(
                                "p (h d) -> p h d", h=NHL),
                            bv_sb[:].rearrange("p (h d) -> p h d", h=NHL),
                        )
                        nc.vector.memset(vt[:, :, HS:HS + 1], 1.0)

                return [lambda c=c: unit(c) for c in range(HCHUNKS)]

            combt = [combtp.tile([128, S], bf16, tag=f"ct{hp}", name=f"ct{hp}")
                     for hp in range(NHP)]
            # partial output accumulator [128, st, 768] (bf16: host sums
            # the two per-batch partials in fp32)
            out_acc = oaccp.tile([128, S // 128, H], bf16, tag="oacc")

            def emit_qkt(hp):
                """Q^T/K^T projection for head pair hp; returns (qt, kt, units).
                Each unit emits one matmul (plus bias-add drain on the last)."""
                qt_t = qkt.tile([128, S], bf16, tag="qt", name=f"qt{hp}")
                kt_t = qkt.tile([128, S], bf16, tag="kt", name=f"kt{hp}")
                units = []
                chains = {}
                for kind, dst, w_t, b_sb in (("kt", kt_t, wk_t, bk_sb),
                                             ("qt", qt_t, wq_t, bq_sb)):
                    for sq in range(SQT):
                        st8 = {}

                        def unit(c, dst=dst, w_t=w_t, b_sb=b_sb, sq=sq, st8=st8):
                            if c == 0:
                                st8["ps"] = ps_pj.tile(
                                    [128, 512], fp32, tag="pj", name="psq")
                            nc.tensor.matmul(
                                st8["ps"][:],
                                lhsT=w_t[:, c, hp * 128:(hp + 1) * 128],
                                rhs=xt_t[sq][:, c, :],
                                start=(c == 0),
                                stop=(c == HCHUNKS - 1),
                            )
                            if c == HCHUNKS - 1:
                                nc.vector.tensor_scalar_add(
                                    dst[:, sq * QW:(sq + 1) * QW], st8["ps"][:],
                                    b_sb[:, hp:hp + 1],
                                )

                        chain = [lambda c=c, u=unit: u(c)
                                 for c in range(HCHUNKS)]
                        chains[(kind, sq)] = chain
                        units.extend(chain)
                return qt_t, kt_t, units, chains

            def emit_outproj_unit(hp, st, half, stream_out=False):
                """Single K=128 out-proj matmul for (head pair hp, s-tile
                st, 384-col half), contracting both heads at once, plus
                the accumulate drain."""
                ps = ps_pj.tile([128, 512], fp32, tag="pj", name="pso")
                nc.tensor.matmul(
                    ps[:, 0:384],
                    lhsT=combt[hp][:, st * 128:(st + 1) * 128],
                    rhs=wo_sb[hp][:, half * 384:(half + 1) * 384],
                    start=True, stop=True,
                )
                dst = out_acc[:, st, half * 384:(half + 1) * 384]
                if hp == 0:
                    nc.vector.tensor_copy(dst, ps[:, 0:384])
                else:
                    nc.vector.tensor_add(dst, dst, ps[:, 0:384])
                if stream_out:
                    nc.sync.dma_start(
                        out_d[st * 128:(st + 1) * 128, :], out_acc[:, st, :])

            def outproj_units(hp, sqs, stream_out=False, min_sq=None):
                """(min_sq, unit) out-proj work for the s-tiles inside query
                tiles `sqs` of head pair hp, gated one sq later (or at an
                explicit min_sq when queued into a later head pair)."""
                units = []
                for sq in sqs:
                    gate = sq + 2 if min_sq is None else min_sq
                    for st in range(4 * sq, 4 * (sq + 1)):
                        for half in range(2):
                            units.append((gate, lambda hp=hp, st=st, half=half,
                                          so=stream_out and half == 1:
                                          emit_outproj_unit(hp, st, half, so)))
                return units

            slots = [(hp, sq, kt) for hp in range(NHP) for sq in range(SQT)
                     for kt in range(SKT)]

            def scores(hp, sq, kt):
                qt_t, kt_t = qkts[hp][0], qkts[hp][1]
                sc = ps_sc.tile([128, 1024], fp32, tag="sc", name="sc")
                nc.tensor.matmul(
                    sc[:, 0:512],
                    lhsT=kt_t[0:64, kt * 128:(kt + 1) * 128],
                    rhs=qt_t[0:64, sq * QW:(sq + 1) * QW],
                    start=True, stop=True,
                )
                nc.tensor.matmul(
                    sc[:, 512:1024],
                    lhsT=kt_t[64:128, kt * 128:(kt + 1) * 128],
                    rhs=qt_t[64:128, sq * QW:(sq + 1) * QW],
                    start=True, stop=True,
                )
                return sc

            # ---- pre-loop: hp0 K/Q chains for sq0, first scores, V kt0
            # (the rest of V is drained slot-by-slot with a one-slot
            # lookahead so every v tile's writers are emitted before the
            # attn@V matmul that reads it — Tile orders by program
            # order, so a late-emitted writer is a silent race) ----
            qkts = [emit_qkt(0)]
            ch0 = qkts[0][3]
            for u in ch0[("kt", 0)] + ch0[("qt", 0)]:
                u()
            sc_cur = scores(*slots[0])
            for u in v_units(0):
                u()
            for u in ch0[("kt", 1)]:
                u()
            v_pending = {kt: v_units(kt) for kt in range(1, SKT)}

            # per-hp injection queues: (min_sq, emit_fn)
            inject_q = {0: [], 1: [], 2: []}
            qkts.append(emit_qkt(1))
            inject_q[0] = (
                [(0, u) for u in ch0[("kt", 2)]]
                + [(0, u) for u in ch0[("kt", 3)]]
                + [(0, u) for u in ch0[("qt", 1)]]
                + [(1, u) for u in ch0[("qt", 2)]]
                + [(2, u) for u in ch0[("qt", 3)]]
                + [(0, u) for u in qkts[1][2]]
                + outproj_units(0, range(SQT - 2)))

            cb_cur = None
            for i, (hp, sq, kt) in enumerate(slots):
                if kt == 0:
                    if sq == 0 and hp > 0:
                        # drain any leftover injected work of the previous hp
                        for _, u in inject_q[hp - 1]:
                            u()
                        inject_q[hp - 1] = []
                    # build hp-level injection queues lazily at hp start
                    if sq == 0 and hp == 1:
                        qkts.append(emit_qkt(2))
                        inject_q[1] = (outproj_units(0, [SQT - 2], min_sq=0)
                                       + [(0, u) for u in qkts[2][2]]
                                       + outproj_units(1, range(SQT - 2)))

                    if sq == 0 and hp == 2:
                        inject_q[2] = (outproj_units(1, [SQT - 2], min_sq=0)
                                       + outproj_units(2, range(SQT - 2),
                                                       stream_out=True)
                                       + outproj_units(2, [SQT - 2],
                                                       stream_out=True,
                                                       min_sq=SQT - 1))
                    cb_a = ps_cb.tile([65, 512], fp32, tag="cb", name="cba")
                    cb_b = ps_cb.tile([65, 512], fp32, tag="cb", name="cbb")
                    cb_cur = (cb_a, cb_b)
                # lookahead scores for the next slot
                sc_nxt = scores(*slots[i + 1]) if i + 1 < len(slots) else None
                at = attnp.tile([128, 1024], bf16, tag="at")
                nc.scalar.activation(
                    at[:], sc_cur[:], AF.Exp,
                    bias=mask_sb[:, kt:kt + 1], scale=0.125,
                )
                # during hp0-sq0, emit next slot's V projection (writers
                # must precede the attn@V reader in program order)
                if hp == 0 and sq == 0 and kt + 1 in v_pending:
                    for u in v_pending.pop(kt + 1):
                        u()
                # fill the PE exp-wait bubble with independent work
                # (scan past gated units so a blocked head doesn't starve
                # eligible work behind it)
                q = inject_q[hp]
                popped = 0
                max_pop = 2 if hp == 0 else (3 if (hp == 1 and sq == 0) else 2)
                j = 0
                while j < len(q) and popped < max_pop:
                    if q[j][0] <= sq:
                        q.pop(j)[1]()
                        popped += 1
                    else:
                        j += 1
                cb_a, cb_b = cb_cur
                nc.tensor.matmul(
                    cb_a[:],
                    lhsT=v_sb[kt][:, 2 * hp, :],
                    rhs=at[:, 0:512],
                    start=(kt == 0), stop=(kt == SKT - 1),
                )
                nc.tensor.matmul(
                    cb_b[:],
                    lhsT=v_sb[kt][:, 2 * hp + 1, :],
                    rhs=at[:, 512:1024],
                    start=(kt == 0), stop=(kt == SKT - 1),
                )
                sc_cur = sc_nxt
                if kt == SKT - 1:
                    # normalize: comb rows 0..63 / denom (row 64).
                    # Two quick copies free both PSUM banks before the
                    # slow recip/broadcast chains run.
                    cbs_list = []
                    for cb in (cb_a, cb_b):
                        cbs = smallp.tile([65, 512], fp32, tag="cbs", name="cbs")
                        nc.vector.tensor_copy(cbs[:], cb[:])
                        cbs_list.append(cbs)
                    for half, cbs in ((0, cbs_list[0]), (1, cbs_list[1])):
                        rc0 = smallp.tile([1, 512], fp32, tag="rc0")
                        nc.sync.dma_start(rc0[:], cbs[64:65, :])
                        rc1 = smallp.tile([1, 512], fp32, tag="rc1")
                        # approx recip is partition-0 only on HW
                        nc.vector.reciprocal_approx_fast(rc1[:], rc0[:])
                        bc = smallp.tile([64, 512], fp32, tag="bc")
                        nc.gpsimd.partition_broadcast(bc[:], rc1[:])
                        nc.vector.tensor_mul(
                            combt[hp][64 * half:64 * (half + 1),
                                      sq * QW:(sq + 1) * QW],
                            cbs[0:64, :], bc[:],
                        )

            # ---- tail: leftovers, then sq3 out-proj for all three head
            # pairs as single PSUM-accumulated chains (one DVE copy per
            # (st, half) instead of copy+add+add), streaming out ----
            for hp in range(NHP):
                for _, u in inject_q[hp]:
                    u()
                inject_q[hp] = []
            for st in range(4 * (SQT - 1), 4 * SQT):
                for half in range(2):
                    ps = ps_pj.tile([128, 512], fp32, tag="pj", name="pso")
                    for hp in range(NHP):
                        nc.tensor.matmul(
                            ps[:, 0:384],
                            lhsT=combt[hp][:, st * 128:(st + 1) * 128],
                            rhs=wo_sb[hp][:, half * 384:(half + 1) * 384],
                            start=(hp == 0), stop=(hp == NHP - 1),
                        )
                    nc.vector.tensor_copy(
                        out_acc[:, st, half * 384:(half + 1) * 384],
                        ps[:, 0:384])
                    if half == 1:
                        nc.sync.dma_start(
                            out_d[st * 128:(st + 1) * 128, :],
                            out_acc[:, st, :])

    nc.compile()
    return nc


def _get_compiled():
    global _COMPILED
    if _COMPILED is None:
        _COMPILED = _build()
    return _COMPILED


def _prep_core_inputs(x, mask, Wq, bq, Wk, bk, Wv, bv, Wo, core):
    b, hg = core // 2, core % 2
    lo, hi = hg * NHL * HS, (hg + 1) * NHL * HS
    bf = ml_dtypes.bfloat16
    return {
        "xt": np.ascontiguousarray(x[b].T).astype(bf),
        "wq": np.ascontiguousarray(Wq[:, lo:hi]).astype(bf),
        "wk": np.ascontiguousarray(Wk[:, lo:hi]).astype(bf),
        "wv": np.ascontiguousarray(Wv[:, lo:hi]).astype(bf),
        "wo": np.ascontiguousarray(Wo[lo:hi, :]).astype(bf),
        "bq": np.ascontiguousarray(bq[lo:hi].reshape(NHP, 128).T).astype(np.float32),
        "bk": np.ascontiguousarray(bk[lo:hi].reshape(NHP, 128).T).astype(np.float32),
        "bv": np.tile(bv[lo:hi][None, :], (128, 1)).astype(bf),
        "mask": np.ascontiguousarray(
            mask[b, 0, 0].reshape(SKT, 128).T).astype(np.float32),
    }


def kernel(x, additive_attention_mask, Wq, bq, Wk, bk, Wv, bv, Wo, bo):
    from concourse import bass2jax

    x = np.asarray(x, dtype=np.float32)
    mask = np.asarray(additive_attention_mask, dtype=np.float32)
    args = [np.asarray(a, dtype=np.float32) for a in (Wq, bq, Wk, bk, Wv, bv, Wo)]
    Wq, bq, Wk, bk, Wv, bv, Wo = args
    bo = np.asarray(bo, dtype=np.float32)

    nc = _get_compiled()
    in_maps = [
        _prep_core_inputs(x, mask, Wq, bq, Wk, bk, Wv, bv, Wo, c)
        for c in range(N_CORES)
    ]
    results = bass2jax.run_bass_via_pjrt(nc, in_maps, n_cores=N_CORES)

    out = np.empty((B, S, H), dtype=np.float32)
    for b in range(B):
        out[b] = (results[2 * b]["out"].astype(np.float32)
                  + results[2 * b + 1]["out"].astype(np.float32) + bo)
    return out


# revision 25
# speedup vs baseline: 1.0162x; 1.0115x over previous
"""BERT self-attention (B=4, S=2048, H=768, 12 heads x 64) on 8 trn2 cores.

Sharding: core c = batch (c//2) x head-half (c%2, 6 heads each).
Each core computes Q/K/V projections for its 6 heads, attention, and a
partial output projection (its heads' slice of Wo). Host sums the two
partials per batch and adds bo.

On-device layout (per core):
  xT   [768, 2048]  bf16  (host-transposed x), 4 pieces of 512 per chunk
  QT/KT per head-pair [128=2x64, 2048] bf16   (head-dim on partitions)
  V    16 tiles [128 keys, 6 heads x 65] bf16 (col 0 of each head = 1.0)
  scores^T [128 keys, 2x512 q] fp32 PSUM, two heads concurrent via PE
  row tiling (row_grp h0/h64, shared 512-col rhs stream)
  exp on ScalarE (scale=1/8, bias=mask column), out bf16
  attn@V -> cb [65, 512] PSUM; row 0 = softmax denominator
  combT packed [128, 2048] bf16 per head pair, scaled by 1/denom
  out-proj: single K=128 matmul per (st, half); out_acc bf16, out bf16

The attention loop is ACT(exp)-bound (~1.11us per 128x1024 exp). The
in-order PE is kept fed by (a) pipelining the score matmuls one slot
ahead and (b) injecting independent projection matmuls into the
exp-wait bubble between scores(i+1) and attn@V(i). Startup DMAs issue
on the sync+gpsimd queues (scalar queue stays clear so exp(0) is not
delayed behind DMA issue costs), ordered so the hp0 K/Q chains start
as early as possible.
"""

import numpy as np
import ml_dtypes

B, S, H = 4, 2048, 768
NH, HS = 12, 64
NHL = 6              # heads per core
NHP = 3              # head pairs per core
HCHUNKS = 6          # 768 / 128 contraction chunks
SKT = 16             # key tiles of 128
SQT = 4              # query tiles of 512
QW = 512             # query tile width
N_CORES = 8

_COMPILED = None


def _build():
    import concourse.bass as bass
    import concourse.mybir as mybir
    import concourse.tile as tile
    from concourse import bacc

    fp32 = mybir.dt.float32
    bf16 = mybir.dt.bfloat16
    AF = mybir.ActivationFunctionType

    nc = bacc.Bacc("TRN2", target_bir_lowering=False, debug=False)

    # host pre-packs x^T and the weights into [128, wide-row] blocks so
    # each tensor is one large contiguous DMA (per-queue DMA pacing is
    # ~1.4us per transfer regardless of size, so fewer/bigger wins)
    xtp_d = [nc.dram_tensor(f"xt{p}", [128, HCHUNKS, QW], bf16,
                            kind="ExternalInput").ap() for p in range(SQT)]
    wq_d = nc.dram_tensor("wq", [128, HCHUNKS, NHL * HS], bf16,
                          kind="ExternalInput").ap()
    wk_d = nc.dram_tensor("wk", [128, HCHUNKS, NHL * HS], bf16,
                          kind="ExternalInput").ap()
    wv_d = nc.dram_tensor("wv", [128, HCHUNKS, NHL * HS], bf16,
                          kind="ExternalInput").ap()
    wo_d = nc.dram_tensor("wo", [128, NHP, H], bf16,
                          kind="ExternalInput").ap()
    bq_d = nc.dram_tensor("bq", [128, NHP], fp32, kind="ExternalInput").ap()
    bk_d = nc.dram_tensor("bk", [128, NHP], fp32, kind="ExternalInput").ap()
    bv_d = nc.dram_tensor("bv", [128, NHL * HS], bf16, kind="ExternalInput").ap()
    mask_d = nc.dram_tensor("mask", [128, SKT], fp32, kind="ExternalInput").ap()
    out_d = nc.dram_tensor("out", [S, H], bf16, kind="ExternalOutput").ap()

    with tile.TileContext(nc) as tc:
        with (
            tc.tile_pool(name="const", bufs=1) as const,
            tc.tile_pool(name="xt", bufs=1) as xtp,
            tc.tile_pool(name="vsb", bufs=1) as vsb,
            tc.tile_pool(name="qkt", bufs=2) as qkt,
            tc.tile_pool(name="combt", bufs=1) as combtp,
            tc.tile_pool(name="oacc", bufs=1) as oaccp,
            tc.tile_pool(name="attn", bufs=5) as attnp,
            tc.tile_pool(name="small", bufs=4) as smallp,
            tc.tile_pool(name="ps_sc", bufs=2, space="PSUM") as ps_sc,
            tc.tile_pool(name="ps_cb", bufs=2, space="PSUM") as ps_cb,
            tc.tile_pool(name="ps_pj", bufs=2, space="PSUM") as ps_pj,
        ):
            # ---- startup DMAs in consumption order. The hp0 K chain
            # needs (xt piece0, wk), the Q chain wq; V needs wv + xt
            # piece1. Issues cost ~650ns of queue time each and pace
            # the transfers, so split by need-time: sync carries
            # xt0/wv/xt2/xt3/wo, scalar carries wk/wq/biases/xt1 and
            # clears its backlog (~14us) before the first exp. ----
            xt_t = [xtp.tile([128, HCHUNKS, QW], bf16, tag=f"xt{p}",
                             name=f"xt{p}") for p in range(SQT)]
            wk_t = const.tile([128, HCHUNKS, NHL * HS], bf16, tag="wk")
            wq_t = const.tile([128, HCHUNKS, NHL * HS], bf16, tag="wq")
            wv_t = const.tile([128, HCHUNKS, NHL * HS], bf16, tag="wv")
            bq_sb = const.tile([128, NHP], fp32, tag="bq")
            bk_sb = const.tile([128, NHP], fp32, tag="bk")
            bv_sb = const.tile([128, NHL * HS], bf16, tag="bv")
            mask_sb = const.tile([128, SKT], fp32, tag="mask")
            wo_t = const.tile([128, NHP, H], bf16, tag="wo")

            # critical tensors on three different queues, in
            # consumption order
            nc.sync.dma_start(xt_t[0][:], xtp_d[0])
            nc.scalar.dma_start(wk_t[:], wk_d)
            nc.scalar.dma_start(bk_sb[:], bk_d[:])
            nc.gpsimd.dma_start(wq_t[:], wq_d)
            nc.sync.dma_start(wv_t[:], wv_d)
            nc.scalar.dma_start(bq_sb[:], bq_d[:])
            nc.scalar.dma_start(mask_sb[:], mask_d[:])
            nc.scalar.dma_start(bv_sb[:], bv_d[:])
            nc.scalar.dma_start(wo_t[:], wo_d)
            nc.gpsimd.dma_start(xt_t[3][:], xtp_d[3])
            nc.sync.dma_start(xt_t[1][:], xtp_d[1])
            nc.gpsimd.dma_start(xt_t[2][:], xtp_d[2])

            # ---- V projection: V[s, h*65 + 1 + d], col h*65+0 = 1.0
            # (denominator lands on PSUM partition 0 of the attn@V
            # output, where reciprocal can read it directly).
            # Emitted as per-kt unit chains so the tail can be injected
            # into the attention loop. ----
            v_sb = [vsb.tile([128, NHL, HS + 1], bf16, tag=f"v{kt}", name=f"v{kt}")
                    for kt in range(SKT)]

            def v_units(kt):
                vt = v_sb[kt]
                st8 = {}

                def unit(c, st8=st8):
                    if c == 0:
                        st8["ps"] = ps_pj.tile([128, 512], fp32, tag="pj",
                                               name="psv")
                    nc.tensor.matmul(
                        st8["ps"][:, :NHL * HS],
                        lhsT=xt_t[kt // 4][:, c,
                                           (kt % 4) * 128:(kt % 4 + 1) * 128],
                        rhs=wv_t[:, c, :],
                        start=(c == 0),
                        stop=(c == HCHUNKS - 1),
                    )
                    if c == HCHUNKS - 1:
                        nc.vector.tensor_add(
                            vt[:, :, 0:HS],
                            st8["ps"][:, :NHL * HS].rearrange(
                                "p (h d) -> p h d", h=NHL),
                            bv_sb[:].rearrange("p (h d) -> p h d", h=NHL),
                        )
                        nc.vector.memset(vt[:, :, HS:HS + 1], 1.0)

                return [lambda c=c: unit(c) for c in range(HCHUNKS)]

            combt = [combtp.tile([128, S], bf16, tag=f"ct{hp}", name=f"ct{hp}")
                     for hp in range(NHP)]
            # partial output accumulator [128, st, 768] (bf16: host sums
            # the two per-batch partials in fp32)
            out_acc = oaccp.tile([128, S // 128, H], bf16, tag="oacc")

            def emit_qkt(hp):
                """Q^T/K^T projection for head pair hp; returns (qt, kt, units).
                Each unit emits one matmul (plus bias-add drain on the last)."""
                qt_t = qkt.tile([128, S], bf16, tag="qt", name=f"qt{hp}")
                kt_t = qkt.tile([128, S], bf16, tag="kt", name=f"kt{hp}")
                units = []
                chains = {}
                for kind, dst, w_t, b_sb in (("kt", kt_t, wk_t, bk_sb),
                                             ("qt", qt_t, wq_t, bq_sb)):
                    for sq in range(SQT):
                        st8 = {}

                        def unit(c, dst=dst, w_t=w_t, b_sb=b_sb, sq=sq, st8=st8):
                            if c == 0:
                                st8["ps"] = ps_pj.tile(
                                    [128, 512], fp32, tag="pj", name="psq")
                            nc.tensor.matmul(
                                st8["ps"][:],
                                lhsT=w_t[:, c, hp * 128:(hp + 1) * 128],
                                rhs=xt_t[sq][:, c, :],
                                start=(c == 0),
                                stop=(c == HCHUNKS - 1),
                            )
                            if c == HCHUNKS - 1:
                                nc.vector.tensor_scalar_add(
                                    dst[:, sq * QW:(sq + 1) * QW], st8["ps"][:],
                                    b_sb[:, hp:hp + 1],
                                )

                        chain = [lambda c=c, u=unit: u(c)
                                 for c in range(HCHUNKS)]
                        chains[(kind, sq)] = chain
                        units.extend(chain)
                return qt_t, kt_t, units, chains

            def emit_outproj_unit(hp, st, half, stream_out=False):
                """Single K=128 out-proj matmul for (head pair hp, s-tile
                st, 384-col half), contracting both heads at once, plus
                the accumulate drain."""
                ps = ps_pj.tile([128, 512], fp32, tag="pj", name="pso")
                nc.tensor.matmul(
                    ps[:, 0:384],
                    lhsT=combt[hp][:, st * 128:(st + 1) * 128],
                    rhs=wo_t[:, hp, half * 384:(half + 1) * 384],
                    start=True, stop=True,
                )
                dst = out_acc[:, st, half * 384:(half + 1) * 384]
                if hp == 0:
                    nc.vector.tensor_copy(dst, ps[:, 0:384])
                else:
                    nc.vector.tensor_add(dst, dst, ps[:, 0:384])
                if stream_out:
                    nc.sync.dma_start(
                        out_d[st * 128:(st + 1) * 128, :], out_acc[:, st, :])

            def outproj_units(hp, sqs, stream_out=False, min_sq=None):
                """(min_sq, unit) out-proj work for the s-tiles inside query
                tiles `sqs` of head pair hp, gated one sq later (or at an
                explicit min_sq when queued into a later head pair)."""
                units = []
                for sq in sqs:
                    gate = sq + 2 if min_sq is None else min_sq
                    for st in range(4 * sq, 4 * (sq + 1)):
                        for half in range(2):
                            units.append((gate, lambda hp=hp, st=st, half=half,
                                          so=stream_out and half == 1:
                                          emit_outproj_unit(hp, st, half, so)))
                return units

            slots = [(hp, sq, kt) for hp in range(NHP) for sq in range(SQT)
                     for kt in range(SKT)]

            def scores(hp, sq, kt):
                qt_t, kt_t = qkts[hp][0], qkts[hp][1]
                sc = ps_sc.tile([128, 1024], fp32, tag="sc", name="sc")
                nc.tensor.matmul(
                    sc[:, 0:512],
                    lhsT=kt_t[0:64, kt * 128:(kt + 1) * 128],
                    rhs=qt_t[0:64, sq * QW:(sq + 1) * QW],
                    start=True, stop=True,
                )
                nc.tensor.matmul(
                    sc[:, 512:1024],
                    lhsT=kt_t[64:128, kt * 128:(kt + 1) * 128],
                    rhs=qt_t[64:128, sq * QW:(sq + 1) * QW],
                    start=True, stop=True,
                )
                return sc

            # ---- pre-loop: hp0 K/Q chains for sq0, first scores, V kt0
            # (the rest of V is drained slot-by-slot with a one-slot
            # lookahead so every v tile's writers are emitted before the
            # attn@V matmul that reads it — Tile orders by program
            # order, so a late-emitted writer is a silent race) ----
            qkts = [emit_qkt(0)]
            ch0 = qkts[0][3]
            for u in ch0[("kt", 0)] + ch0[("qt", 0)]:
                u()
            sc_cur = scores(*slots[0])
            for u in v_units(0):
                u()
            for u in ch0[("kt", 1)]:
                u()
            v_pending = {kt: v_units(kt) for kt in range(1, SKT)}

            # per-hp injection queues: (min_sq, emit_fn)
            inject_q = {0: [], 1: [], 2: []}
            qkts.append(emit_qkt(1))
            inject_q[0] = (
                [(0, u) for u in ch0[("kt", 2)]]
                + [(0, u) for u in ch0[("kt", 3)]]
                + [(0, u) for u in ch0[("qt", 1)]]
                + [(1, u) for u in ch0[("qt", 2)]]
                + [(2, u) for u in ch0[("qt", 3)]]
                + [(0, u) for u in qkts[1][2]]
                + outproj_units(0, range(SQT - 2)))

            cb_cur = None
            for i, (hp, sq, kt) in enumerate(slots):
                if kt == 0:
                    if sq == 0 and hp > 0:
                        # drain any leftover injected work of the previous hp
                        for _, u in inject_q[hp - 1]:
                            u()
                        inject_q[hp - 1] = []
                    # build hp-level injection queues lazily at hp start
                    if sq == 0 and hp == 1:
                        qkts.append(emit_qkt(2))
                        inject_q[1] = (outproj_units(0, [SQT - 2], min_sq=0)
                                       + [(0, u) for u in qkts[2][2]]
                                       + outproj_units(1, range(SQT - 2)))

                    if sq == 0 and hp == 2:
                        inject_q[2] = (outproj_units(1, [SQT - 2], min_sq=0)
                                       + outproj_units(2, range(SQT - 2),
                                                       stream_out=True)
                                       + outproj_units(2, [SQT - 2],
                                                       stream_out=True,
                                                       min_sq=SQT - 1))
                    cb_a = ps_cb.tile([65, 512], fp32, tag="cb", name="cba")
                    cb_b = ps_cb.tile([65, 512], fp32, tag="cb", name="cbb")
                    cb_cur = (cb_a, cb_b)
                # lookahead scores for the next slot
                sc_nxt = scores(*slots[i + 1]) if i + 1 < len(slots) else None
                at = attnp.tile([128, 1024], bf16, tag="at")
                nc.scalar.activation(
                    at[:], sc_cur[:], AF.Exp,
                    bias=mask_sb[:, kt:kt + 1], scale=0.125,
                )
                # during hp0-sq0, emit next slot's V projection (writers
                # must precede the attn@V reader in program order)
                if hp == 0 and sq == 0 and kt + 1 in v_pending:
                    for u in v_pending.pop(kt + 1):
                        u()
                # fill the PE exp-wait bubble with independent work
                # (scan past gated units so a blocked head doesn't starve
                # eligible work behind it)
                q = inject_q[hp]
                popped = 0
                max_pop = 2 if hp == 0 else (3 if (hp == 1 and sq == 0) else 2)
                j = 0
                while j < len(q) and popped < max_pop:
                    if q[j][0] <= sq:
                        q.pop(j)[1]()
                        popped += 1
                    else:
                        j += 1
                cb_a, cb_b = cb_cur
                nc.tensor.matmul(
                    cb_a[:],
                    lhsT=v_sb[kt][:, 2 * hp, :],
                    rhs=at[:, 0:512],
                    start=(kt == 0), stop=(kt == SKT - 1),
                )
                nc.tensor.matmul(
                    cb_b[:],
                    lhsT=v_sb[kt][:, 2 * hp + 1, :],
                    rhs=at[:, 512:1024],
                    start=(kt == 0), stop=(kt == SKT - 1),
                )
                sc_cur = sc_nxt
                if kt == SKT - 1:
                    # normalize: comb rows 0..63 / denom (row 64).
                    # Two quick copies free both PSUM banks before the
                    # slow recip/broadcast chains run.
                    cbs_list = []
                    for cb in (cb_a, cb_b):
                        cbs = smallp.tile([65, 512], fp32, tag="cbs", name="cbs")
                        nc.vector.tensor_copy(cbs[:], cb[:])
                        cbs_list.append(cbs)
                    for half, cbs in ((0, cbs_list[0]), (1, cbs_list[1])):
                        rc0 = smallp.tile([1, 512], fp32, tag="rc0")
                        nc.sync.dma_start(rc0[:], cbs[64:65, :])
                        rc1 = smallp.tile([1, 512], fp32, tag="rc1")
                        # approx recip is partition-0 only on HW
                        nc.vector.reciprocal_approx_fast(rc1[:], rc0[:])
                        bc = smallp.tile([64, 512], fp32, tag="bc")
                        nc.gpsimd.partition_broadcast(bc[:], rc1[:])
                        nc.vector.tensor_mul(
                            combt[hp][64 * half:64 * (half + 1),
                                      sq * QW:(sq + 1) * QW],
                            cbs[0:64, :], bc[:],
                        )

            # ---- tail: leftovers, then sq3 out-proj for all three head
            # pairs as single PSUM-accumulated chains (one DVE copy per
            # (st, half) instead of copy+add+add), streaming out ----
            for hp in range(NHP):
                for _, u in inject_q[hp]:
                    u()
                inject_q[hp] = []
            for st in range(4 * (SQT - 1), 4 * SQT):
                for half in range(2):
                    ps = ps_pj.tile([128, 512], fp32, tag="pj", name="pso")
                    for hp in range(NHP):
                        nc.tensor.matmul(
                            ps[:, 0:384],
                            lhsT=combt[hp][:, st * 128:(st + 1) * 128],
                            rhs=wo_t[:, hp, half * 384:(half + 1) * 384],
                            start=(hp == 0), stop=(hp == NHP - 1),
                        )
                    nc.vector.tensor_copy(
                        out_acc[:, st, half * 384:(half + 1) * 384],
                        ps[:, 0:384])
                    if half == 1:
                        nc.sync.dma_start(
                            out_d[st * 128:(st + 1) * 128, :],
                            out_acc[:, st, :])

    nc.compile()
    return nc


def _get_compiled():
    global _COMPILED
    if _COMPILED is None:
        _COMPILED = _build()
    return _COMPILED


def _prep_core_inputs(x, mask, Wq, bq, Wk, bk, Wv, bv, Wo, core):
    b, hg = core // 2, core % 2
    lo, hi = hg * NHL * HS, (hg + 1) * NHL * HS
    bf = ml_dtypes.bfloat16

    def chunked(w):  # [768, D] -> [128, HCHUNKS, D]
        return np.ascontiguousarray(
            w.reshape(HCHUNKS, 128, -1).transpose(1, 0, 2)).astype(bf)

    xt = x[b].T.reshape(HCHUNKS, 128, S)  # [c][p][s]
    im = {
        "wq": chunked(Wq[:, lo:hi]),
        "wk": chunked(Wk[:, lo:hi]),
        "wv": chunked(Wv[:, lo:hi]),
        "wo": np.ascontiguousarray(
            Wo[lo:hi, :].reshape(NHP, 128, H).transpose(1, 0, 2)).astype(bf),
        "bq": np.ascontiguousarray(bq[lo:hi].reshape(NHP, 128).T).astype(np.float32),
        "bk": np.ascontiguousarray(bk[lo:hi].reshape(NHP, 128).T).astype(np.float32),
        "bv": np.tile(bv[lo:hi][None, :], (128, 1)).astype(bf),
        "mask": np.ascontiguousarray(
            mask[b, 0, 0].reshape(SKT, 128).T).astype(np.float32),
    }
    for p in range(SQT):
        im[f"xt{p}"] = np.ascontiguousarray(
            xt[:, :, p * QW:(p + 1) * QW].transpose(1, 0, 2)).astype(bf)
    return im


def kernel(x, additive_attention_mask, Wq, bq, Wk, bk, Wv, bv, Wo, bo):
    from concourse import bass2jax

    x = np.asarray(x, dtype=np.float32)
    mask = np.asarray(additive_attention_mask, dtype=np.float32)
    args = [np.asarray(a, dtype=np.float32) for a in (Wq, bq, Wk, bk, Wv, bv, Wo)]
    Wq, bq, Wk, bk, Wv, bv, Wo = args
    bo = np.asarray(bo, dtype=np.float32)

    nc = _get_compiled()
    in_maps = [
        _prep_core_inputs(x, mask, Wq, bq, Wk, bk, Wv, bv, Wo, c)
        for c in range(N_CORES)
    ]
    results = bass2jax.run_bass_via_pjrt(nc, in_maps, n_cores=N_CORES)

    out = np.empty((B, S, H), dtype=np.float32)
    for b in range(B):
        out[b] = (results[2 * b]["out"].astype(np.float32)
                  + results[2 * b + 1]["out"].astype(np.float32) + bo)
    return out
